# revision 3
# baseline (speedup 1.0000x reference)
"""AttentionBlock Trainium2 kernel.

Reference computation (B=16, C=512, H=W=32, n_heads=4, d_k=128):
    xs   = x.reshape(B,C,S).T            # [B, S, C],  S = 1024
    qkv  = xs @ w_proj.T + b_proj        # [B, S, 1536], feature f = h*384 + {q:0..128, k:128..256, v:256..384}
    S_   = einsum('bihd,bjhd->bijh', q, k) * d_k**-0.5
    attn = softmax(S_, axis=1)           # over the QUERY axis i (source quirk)
    res  = einsum('bijh,bjhd->bihd', attn, v)
    out  = res @ w_out.T + b_out + xs    # residual
    return out.T.reshape(B, C, H, W)

Strategy: data-parallel over batch, 2 batches per core on 8 cores. Per batch
everything is computed in "transposed" layouts so no on-device transposes are
needed:
  QK^T proj:  psum[f_tile, s] = w_qkT[c, f_tile].T @ x[c, s]      (Q^T/K^T as [d, s])
  V proj:     psum[s_tile, f] = x[c, s_tile].T @ w_vT[c, f]       (V as [s, d])
  scores:     psum[j, i]      = KT[d, j_tile].T @ QT[d, i]        (S^T: softmax axis i = free axis)
  exp+sum:    ACT Exp with scale=d_k**-0.5 and accum_out → row sums over i
  AV:         psum[d, i]     += (V[j_tile, d] * 1/sum[j]).T-style (normalizer folded into V rows)
  out proj:   psum[c_tile, s] = w_outT[f, c_tile].T @ resT[f, s]  (+ b_out + x residual)

All matmul operands are written as float32r (TF32-like, 1 PE cycle/row at
N=512 vs 4 for fp32; measured rel err ~2e-4). PSUM accumulation is fp32.
"""
import sys

for _p in (
    "/opt/trn_rl_repo",
    "/root/.axon_site",
    "/root/.axon_site/_ro/trn_rl_repo",
    "/root/.axon_site/_ro/pypackages",
):
    if _p not in sys.path:
        sys.path.append(_p)

import numpy as np

B = 16
C = 512
S = 1024  # H*W
NH = 4
DK = 128
F = NH * DK  # 512
NCORES = 8
BL = B // NCORES  # batches per core
KT = C // 128  # 4  contraction tiles over channels
ST = S // 128  # 8  seq tiles
NT = S // 512  # 2  free-dim chunks of 512
SCALE = float(DK) ** -0.5

_CACHE: dict = {}


def _build(repeat=1, unroll=1):
    """Build the kernel. repeat>1 wraps the whole per-call workload in an
    on-device For_i loop — used only for timing (one NEFF execution then runs
    the workload `repeat` times, amortizing the ~10ms axon dispatch).
    unroll>1 emits the workload N times inline (cost-model analysis only)."""
    import contextlib

    import concourse.tile as tile
    from concourse import bacc, mybir

    F32 = mybir.dt.float32
    F32R = mybir.dt.float32r
    EXP = mybir.ActivationFunctionType.Exp
    COPY = mybir.ActivationFunctionType.Copy
    IDENT = mybir.ActivationFunctionType.Identity

    nc = bacc.Bacc("TRN2", debug=False)
    x_d = nc.dram_tensor("x", [BL, C, S], F32, kind="ExternalInput").ap()
    wqk_d = nc.dram_tensor("w_qkT", [C, 2 * F], F32, kind="ExternalInput").ap()
    wv_d = nc.dram_tensor("w_vT", [C, F], F32, kind="ExternalInput").ap()
    wo_d = nc.dram_tensor("w_outT", [F, C], F32, kind="ExternalInput").ap()
    bias_d = nc.dram_tensor("bias", [128, 2 * NH + 2 * F + KT], F32, kind="ExternalInput").ap()
    out_d = nc.dram_tensor("out", [BL, C, S], F32, kind="ExternalOutput").ap()

    xr = x_d.bitcast(F32R)
    wqk_r = wqk_d.rearrange("(k p) m -> p k m", p=128).bitcast(F32R)
    wv_r = wv_d.rearrange("(k p) m -> p k m", p=128).bitcast(F32R)
    wo_r = wo_d.rearrange("(k p) m -> p k m", p=128).bitcast(F32R)

    with tile.TileContext(nc) as tc:
        with (
            tc.tile_pool(name="const", bufs=1) as constp,
            tc.tile_pool(name="xp", bufs=2) as xp,
            tc.tile_pool(name="qkp", bufs=2) as qkp,
            tc.tile_pool(name="vp", bufs=1) as vp,
            tc.tile_pool(name="ep", bufs=3) as ep,
            tc.tile_pool(name="rp", bufs=1) as rp,
            tc.tile_pool(name="op", bufs=2) as op,
            tc.tile_pool(name="small", bufs=16) as smallp,
            tc.tile_pool(name="vs", bufs=8) as vsp,
            # psum: pp = [128,512]x2 for qk/v projections; ps = [128,1024]x2
            # for scores and (phase-disjoint) out-projection; pr = [128,512]x2
            # for the per-head AV accumulators. 2+4+2 = 8 banks.
            tc.tile_pool(name="pp", bufs=2, space="PSUM") as pp,
            tc.tile_pool(name="ps", bufs=2, space="PSUM") as ps,
            tc.tile_pool(name="pr", bufs=2, space="PSUM") as pr,
        ):
            # ---- constants ----
            wqk_sb = constp.tile([128, KT, 2 * F], F32R)  # (c_part, c_tile, f_col)
            wv_sb = constp.tile([128, KT, F], F32R)
            wo_sb = constp.tile([128, KT, C], F32R)
            bias_sb = constp.tile([128, 2 * NH + 2 * F + KT], F32)
            # DMA order: x[0] and wqk chunks first (they gate the first
            # matmuls) interleaved across HWDGE queues; wo (only needed at
            # out-proj) last.
            x_sbs = [xp.tile([128, KT, S], F32R, name=f"x{b}", tag="x") for b in range(BL)]
            for k in range(KT):
                nc.sync.dma_start(
                    out=x_sbs[0][:, k, 0:512], in_=xr[0, bass_ts(k, 128), 0:512]
                )
                nc.sync.dma_start(out=wqk_sb[:, k, 0:512], in_=wqk_r[:, k, 0:512])
            for k in range(KT):
                nc.sync.dma_start(
                    out=x_sbs[0][:, k, 512:S], in_=xr[0, bass_ts(k, 128), 512:S]
                )
                nc.sync.dma_start(
                    out=wqk_sb[:, k, 512 : 2 * F], in_=wqk_r[:, k, 512 : 2 * F]
                )
            nc.sync.dma_start(out=wv_sb, in_=wv_r)
            nc.sync.dma_start(out=bias_sb, in_=bias_d)
            for b in range(1, BL):
                for k in range(KT):
                    nc.sync.dma_start(out=x_sbs[b][:, k, :], in_=xr[b, bass_ts(k, 128), :])
            nc.sync.dma_start(out=wo_sb, in_=wo_r)
            b_qk = bias_sb[:, 0 : 2 * NH]  # per-partition bias per qk f-tile
            b_v2 = bias_sb[:, 2 * NH : 2 * NH + 2 * F]  # v bias doubled [128, 2F]
            b_out = bias_sb[:, 2 * NH + 2 * F :]  # per-partition bias per c-tile

            rep_ctx = (
                tc.For_i(0, repeat, 1) if repeat > 1 else contextlib.nullcontext()
            )
            with rep_ctx:
                for _u in range(unroll):
                    _batches(
                        nc, tc, x_sbs, qkp, vp, ep, rp, op, smallp, vsp, pp, ps, pr,
                        wqk_sb, wv_sb, wo_sb, b_qk, b_v2, b_out, out_d, xr,
                        F32, F32R, EXP, IDENT,
                    )

    nc.compile()
    return nc


def _batches(
    nc, tc, x_sbs, qkp, vp, ep, rp, op, smallp, vsp, pp, ps, pr,
    wqk_sb, wv_sb, wo_sb, b_qk, b_v2, b_out, out_d, xr,
    F32, F32R, EXP, IDENT,
):
    if True:
            for b in range(BL):
                x_sb = x_sbs[b]
                qk_sb = qkp.tile([128, 2 * NH, S], F32R)

                def qk_proj(t, x_sb=x_sb, qk_sb=qk_sb):
                    # Q^T/K^T f-tile t: qk_sb[:, t, s] = w_qkT[:, t].T @ x
                    for n in range(NT):
                        acc = pp.tile([128, 512], F32, name="qkacc", tag="pp")
                        for k in range(KT):
                            nc.tensor.matmul(
                                acc,
                                wqk_sb[:, k, bass_ts(t, 128)],
                                x_sb[:, k, bass_ts(n, 512)],
                                start=(k == 0),
                                stop=(k == KT - 1),
                            )
                        nc.vector.tensor_scalar_add(
                            qk_sb[:, t, bass_ts(n, 512)], acc, b_qk[:, t : t + 1]
                        )

                qk_proj(0)
                qk_proj(1)

                # ---- V projection: v_sb[:, st, f] = V rows s-tile st ----
                v_sb = vp.tile([128, ST, F], F32R)
                for st in range(ST):
                    acc = pp.tile([128, 512], F32, name="vacc", tag="pp")
                    for k in range(KT):
                        nc.tensor.matmul(
                            acc,
                            x_sb[:, k, bass_ts(st, 128)],
                            wv_sb[:, k, :],
                            start=(k == 0),
                            stop=(k == KT - 1),
                        )
                    nc.vector.tensor_add(v_sb[:, st, :], acc, b_v2[:, 0:F])

                # ---- attention per head, with the next head's QK projection
                # emitted right after so its PE work fills the ACT-bound
                # softmax phase ----
                resT_sb = rp.tile([128, NH, S], F32R)  # res^T: (d, head, i)
                for h in range(NH):
                    racc = [pr.tile([128, 512], F32, name=f"racc{n}", tag="racc") for n in range(NT)]
                    for jt in range(ST):
                        e_t = ep.tile([128, S], F32R)
                        ssum = smallp.tile([128, 2], F32, name="ssum", tag="ssum")
                        # scores S^T[j, i] for one j-tile: [128, 1024] PSUM
                        # (2 banks); one exp pass over both halves with the
                        # softmax denominator via accum_out.
                        sacc = ps.tile([128, S], F32, name="sacc", tag="sacc")
                        for n in range(NT):
                            nc.tensor.matmul(
                                sacc[:, bass_ts(n, 512)],
                                qk_sb[:, 2 * h + 1, bass_ts(jt, 128)],
                                qk_sb[:, 2 * h, bass_ts(n, 512)],
                                start=True,
                                stop=True,
                            )
                        nc.scalar.activation(
                            out=e_t,
                            in_=sacc,
                            func=EXP,
                            scale=SCALE,
                            accum_out=ssum[:, 0:1],
                        )
                        nc.vector.reciprocal(ssum[:, 1:2], ssum[:, 0:1])
                        v_sc = vsp.tile([128, DK], F32R)
                        nc.vector.tensor_scalar_mul(
                            v_sc,
                            v_sb[:, jt, bass_ts(h, DK)].bitcast(F32),
                            ssum[:, 1:2],
                        )
                        for n in range(NT):
                            nc.tensor.matmul(
                                racc[n],
                                v_sc,
                                e_t[:, bass_ts(n, 512)],
                                start=(jt == 0),
                                stop=(jt == ST - 1),
                            )
                    for n in range(NT):
                        nc.vector.tensor_copy(
                            resT_sb[:, h, bass_ts(n, 512)], racc[n]
                        )
                    if h + 1 < NH:
                        qk_proj(2 * h + 2)
                        qk_proj(2 * h + 3)

                # ---- output projection + bias + residual ----
                for ct in range(KT):
                    out_t = op.tile([128, S], F32)
                    acc = ps.tile([128, S], F32, name="oacc", tag="sacc")
                    for n in range(NT):
                        for k in range(NH):
                            nc.tensor.matmul(
                                acc[:, bass_ts(n, 512)],
                                wo_sb[:, k, bass_ts(ct, 128)],
                                resT_sb[:, k, bass_ts(n, 512)],
                                start=(k == 0),
                                stop=(k == NH - 1),
                            )
                    # per-half bias+residual+store so the tail drains at 512
                    # granularity (first half's DMA overlaps second half's ops)
                    res_eng = nc.vector if b == BL - 1 else nc.gpsimd
                    for n in range(NT):
                        nc.scalar.activation(
                            out=out_t[:, bass_ts(n, 512)],
                            in_=acc[:, bass_ts(n, 512)],
                            func=IDENT,
                            bias=b_out[:, ct : ct + 1],
                        )
                        res_eng.tensor_add(
                            out_t[:, bass_ts(n, 512)],
                            out_t[:, bass_ts(n, 512)],
                            x_sb[:, ct, bass_ts(n, 512)].bitcast(F32),
                        )
                        nc.sync.dma_start(
                            out=out_d[b, bass_ts(ct, 128), bass_ts(n, 512)],
                            in_=out_t[:, bass_ts(n, 512)],
                        )


def bass_ts(i, size):
    import concourse.bass as bass

    return bass.ts(i, size)


def _prep_inputs(x, w_proj, b_proj, w_out, b_out):
    """Host-side reshaping into the layouts the kernel expects."""
    x_f = np.ascontiguousarray(x.reshape(B, C, S), dtype=np.float32)
    wT = np.asarray(w_proj, dtype=np.float32).T  # [C, 3*F], f = h*384 + j
    w_qkT = np.concatenate(
        [wT[:, h * 384 : h * 384 + 256] for h in range(NH)], axis=1
    )  # [C, 2F]; col tile t=2h -> q_h, t=2h+1 -> k_h
    w_vT = np.concatenate(
        [wT[:, h * 384 + 256 : h * 384 + 384] for h in range(NH)], axis=1
    )  # [C, F]
    w_outT = np.ascontiguousarray(np.asarray(w_out, dtype=np.float32).T)  # [F, C]
    b_proj = np.asarray(b_proj, dtype=np.float32)
    b_qk = np.stack(
        [
            b_proj[h * 384 + half * 128 : h * 384 + half * 128 + 128]
            for h in range(NH)
            for half in range(2)
        ],
        axis=1,
    )  # [128, 2*NH], col t matches qk tile order
    b_v = np.concatenate(
        [b_proj[h * 384 + 256 : h * 384 + 384] for h in range(NH)]
    )  # [F]
    b_v_bcast = np.broadcast_to(np.concatenate([b_v, b_v]), (128, 2 * F))
    b_out_t = np.asarray(b_out, dtype=np.float32).reshape(KT, 128).T  # [128, KT]
    bias = np.ascontiguousarray(
        np.concatenate([b_qk, b_v_bcast, b_out_t], axis=1), dtype=np.float32
    )  # [128, 2*NH + 2*F + KT]
    return x_f, np.ascontiguousarray(w_qkT), np.ascontiguousarray(w_vT), w_outT, bias


def kernel(x, w_proj, b_proj, w_out, b_out, n_heads):
    from concourse.bass_utils import run_bass_kernel_spmd

    assert int(n_heads) == NH
    x_f, w_qkT, w_vT, w_outT, bias = _prep_inputs(x, w_proj, b_proj, w_out, b_out)

    if "nc" not in _CACHE:
        _CACHE["nc"] = _build()
    nc = _CACHE["nc"]

    in_maps = [
        {
            "x": np.ascontiguousarray(x_f[c * BL : (c + 1) * BL]),
            "w_qkT": w_qkT,
            "w_vT": w_vT,
            "w_outT": w_outT,
            "bias": bias,
        }
        for c in range(NCORES)
    ]
    res = run_bass_kernel_spmd(nc, in_maps, list(range(NCORES)))
    out = np.concatenate([res.results[c]["out"] for c in range(NCORES)], axis=0)
    return out.reshape(B, C, 32, 32)



# revision 12
# speedup vs baseline: 1.1890x; 1.1890x over previous
"""AttentionBlock Trainium2 kernel (fp8-DoubleRow edition).

Reference computation (B=16, C=512, H=W=32, n_heads=4, d_k=128):
    xs   = x.reshape(B,C,S).T            # [B, S, C],  S = 1024
    qkv  = xs @ w_proj.T + b_proj        # [B, S, 1536]
    S_   = einsum('bihd,bjhd->bijh', q, k) * d_k**-0.5
    attn = softmax(S_, axis=1)           # over the QUERY axis i (source quirk)
    res  = einsum('bijh,bjhd->bihd', attn, v)
    out  = res @ w_out.T + b_out + xs    # residual
    return out.T.reshape(B, C, H, W)

Strategy: data-parallel over batch, 2 batches per core on 8 cores. Transposed
layouts avoid on-device transposes (see qk/score/AV comments below).

Precision: the output is dominated by the fp32 residual xs (+b_out); the
attention path contributes only ~8% of the output magnitude, so it tolerates
fp8. All projections and the AV contraction run as fp8e4 DoubleRow matmuls
(K=256 per pass, 2 output cols/cycle: measured ~207ns vs 270ns for a bf16
K=128 matmul). Scores stay bf16 (contraction is only d_k=128, DoubleRow can't
help, and it keeps exp() inputs accurate). fp8 range handling:
  wqk *= 16   -> qk_sb holds 16*(q|k) in bf16; exp scale absorbs the 256
  wv  *= 64   -> v_sb holds 64*v; v_sc = 64*v/Z stays in fp8-normal range
  wo  *= 16   -> out-proj psum = 1024*(res@wo.T); final activation applies
                 scale=1/1024 and +b_out in the same instruction
  exp bias=-2 (softmax shift-invariant) keeps e^s in [~0.007, ~40] for fp8
The residual add and bias add remain fp32.

PSUM accumulation groups are emitted bank-interleaved (A,B,A,B) because
back-to-back same-bank accumulating matmuls measure ~+60ns each.
"""
import sys

for _p in (
    "/opt/trn_rl_repo",
    "/root/.axon_site",
    "/root/.axon_site/_ro/trn_rl_repo",
    "/root/.axon_site/_ro/pypackages",
):
    if _p not in sys.path:
        sys.path.append(_p)

import numpy as np

B = 16
C = 512
S = 1024  # H*W
NH = 4
DK = 128
F = NH * DK  # 512
NCORES = 8
BL = B // NCORES  # batches per core
KT = C // 128  # 4  contraction tiles over channels
ST = S // 128  # 8  seq tiles
NT = S // 512  # 2  free-dim chunks of 512
SCALE = float(DK) ** -0.5
WQK_SCALE = 16.0  # host pre-scale on w_qkT/b_qk -> scores psum = 256*s
WV_SCALE = 64.0  # host pre-scale on w_vT/b_v -> racc = 64*res
WO_SCALE = 16.0  # host pre-scale on w_outT -> out psum = 1024*out_attn
ESHIFT = -2.0  # exp(s*scale + ESHIFT): softmax-invariant fp8 range shift

_CACHE: dict = {}


def _build(repeat=1, unroll=1):
    """Build the kernel. repeat>1 wraps the whole per-call workload in an
    on-device For_i loop — used only for timing (one NEFF execution then runs
    the workload `repeat` times, amortizing the ~10ms axon dispatch).
    unroll>1 emits the workload N times inline (cost-model analysis only)."""
    import contextlib

    import concourse.tile as tile
    from concourse import bacc, mybir

    F32 = mybir.dt.float32
    BF16 = mybir.dt.bfloat16
    FP8 = mybir.dt.float8e4
    EXP = mybir.ActivationFunctionType.Exp
    IDENT = mybir.ActivationFunctionType.Identity

    nc = bacc.Bacc("TRN2", debug=False)
    x_d = nc.dram_tensor("x", [BL, C, S], F32, kind="ExternalInput").ap()
    wqk_d = nc.dram_tensor("w_qkT", [C, 2 * F], F32, kind="ExternalInput").ap()
    wv_d = nc.dram_tensor("w_vT", [C, F], F32, kind="ExternalInput").ap()
    wo_d = nc.dram_tensor("w_outT", [F, C], F32, kind="ExternalInput").ap()
    bias_d = nc.dram_tensor("bias", [128, 2 * NH + 2 * F + KT + 1], F32, kind="ExternalInput").ap()
    out_d = nc.dram_tensor("out", [BL, C, S], F32, kind="ExternalOutput").ap()

    wqk_r = wqk_d.rearrange("(k p) m -> p k m", p=128)
    wv_r = wv_d.rearrange("(k p) m -> p k m", p=128)
    wo_r = wo_d.rearrange("(k p) m -> p k m", p=128)

    with tile.TileContext(nc) as tc:
        with (
            tc.tile_pool(name="const", bufs=1) as constp,
            tc.tile_pool(name="stage", bufs=1) as stagep,
            tc.tile_pool(name="xp", bufs=2) as xp,
            tc.tile_pool(name="qkp", bufs=2) as qkp,
            tc.tile_pool(name="vp", bufs=2) as vp,
            tc.tile_pool(name="ep", bufs=3) as ep,
            tc.tile_pool(name="rp", bufs=1) as rp,
            tc.tile_pool(name="op", bufs=2) as op,
            tc.tile_pool(name="small", bufs=16) as smallp,
            tc.tile_pool(name="vs", bufs=8) as vsp,
            # psum: pp = [128,512]x2 for qk/v projections; ps = [128,1024]x2
            # for scores and (phase-disjoint) out-projection; pr = [128,512]x2
            # for the per-head AV accumulators. 2+4+2 = 8 banks.
            tc.tile_pool(name="pp", bufs=2, space="PSUM") as pp,
            tc.tile_pool(name="ps", bufs=2, space="PSUM") as ps,
            tc.tile_pool(name="pr", bufs=2, space="PSUM") as pr,
        ):
            # ---- constants: load fp32, convert once to fp8 ----
            wqk8_sb = constp.tile([128, KT, 2 * F], FP8)
            wv8_sb = constp.tile([128, KT, F], FP8)
            wo8_sb = constp.tile([128, KT, C], FP8)
            bias_sb = constp.tile([128, 2 * NH + 2 * F + KT + 1], F32)
            x_sbs = [xp.tile([128, KT, S], F32, name=f"x{b}", tag="x") for b in range(BL)]
            x8_sbs = [xp.tile([128, KT, S], FP8, name=f"x8{b}", tag="x8") for b in range(BL)]
            # xb = x + b_out (residual + out bias pre-added once; the out-proj
            # epilogue is then a single scalar_tensor_tensor per half)
            xb_sbs = [xp.tile([128, KT, S], F32, name=f"xb{b}", tag="xb") for b in range(BL)]

            wqk_st = stagep.tile([128, KT, 2 * F], F32, name="wst", tag="wst")
            for k in range(KT):
                nc.sync.dma_start(out=x_sbs[0][:, k, :], in_=x_d[0, bass_ts(k, 128), :])
                nc.sync.dma_start(out=wqk_st[:, k, :], in_=wqk_r[:, k, :])
            nc.vector.tensor_copy(x8_sbs[0], x_sbs[0])
            nc.gpsimd.tensor_copy(wqk8_sb, wqk_st)
            wv_st = stagep.tile([128, KT, F], F32, name="wst", tag="wst")
            nc.sync.dma_start(out=wv_st, in_=wv_r)
            nc.gpsimd.tensor_copy(wv8_sb, wv_st)
            nc.sync.dma_start(out=bias_sb, in_=bias_d)
            for b in range(1, BL):
                for k in range(KT):
                    nc.sync.dma_start(out=x_sbs[b][:, k, :], in_=x_d[b, bass_ts(k, 128), :])
                nc.vector.tensor_copy(x8_sbs[b], x_sbs[b])
            wo_st = stagep.tile([128, KT, C], F32, name="wst", tag="wst")
            nc.sync.dma_start(out=wo_st, in_=wo_r)
            nc.gpsimd.tensor_copy(wo8_sb, wo_st)
            for b in range(BL):
                for k in range(KT):
                    nc.gpsimd.tensor_scalar_add(
                        xb_sbs[b][:, k, :],
                        x_sbs[b][:, k, :],
                        bias_sb[:, 2 * NH + 2 * F + k : 2 * NH + 2 * F + k + 1],
                    )
            b_qk = bias_sb[:, 0 : 2 * NH]  # per-partition bias per qk f-tile (x16)
            b_v2 = bias_sb[:, 2 * NH : 2 * NH + 2 * F]  # v bias doubled (x64)
            b_out = bias_sb[:, 2 * NH + 2 * F : 2 * NH + 2 * F + KT]  # per-partition bias per c-tile
            b_esh = bias_sb[:, 2 * NH + 2 * F + KT :]  # ESHIFT constant column

            rep_ctx = (
                tc.For_i(0, repeat, 1) if repeat > 1 else contextlib.nullcontext()
            )
            with rep_ctx:
                for _u in range(unroll):
                    _batches(
                        nc, tc, x_sbs, x8_sbs, xb_sbs, qkp, vp, ep, rp, op,
                        smallp, vsp, pp, ps, pr, wqk8_sb, wv8_sb, wo8_sb,
                        b_qk, b_v2, b_out, b_esh, out_d, F32, BF16, FP8, EXP, IDENT,
                    )

    nc.compile()
    return nc


def _batches(
    nc, tc, x_sbs, x8_sbs, xb_sbs, qkp, vp, ep, rp, op, smallp, vsp, pp, ps, pr,
    wqk8_sb, wv8_sb, wo8_sb, b_qk, b_v2, b_out, b_esh, out_d,
    F32, BF16, FP8, EXP, IDENT,
):
    from concourse import mybir

    for b in range(BL):
        x_sb = x_sbs[b]
        x8 = x8_sbs[b]
        xb = xb_sbs[b]
        qk_sb = qkp.tile([128, 2 * NH, S], BF16)

        def qk_proj(t, t2, qk_sb=qk_sb, x8=x8):
            # Q^T/K^T f-tiles t,t2: qk_sb[:, t, s] = 16*w_qkT[:, t].T @ x
            # Two DoubleRow k-blocks (channels 0:256, 256:512), A/B bank
            # interleaved so same-bank accumulating MMs are 2 apart.
            for n in range(NT):
                acc_a = pp.tile([128, 512], F32, name="qka", tag="pp")
                acc_b = pp.tile([128, 512], F32, name="qkb", tag="pp")
                for k2 in range(2):
                    nc.tensor.matmul(
                        acc_a,
                        wqk8_sb[:, 2 * k2 : 2 * k2 + 2, bass_ts(t, 128)],
                        x8[:, 2 * k2 : 2 * k2 + 2, bass_ts(n, 512)],
                        start=(k2 == 0),
                        stop=(k2 == 1),
                        perf_mode=_dr(),
                    )
                    nc.tensor.matmul(
                        acc_b,
                        wqk8_sb[:, 2 * k2 : 2 * k2 + 2, bass_ts(t2, 128)],
                        x8[:, 2 * k2 : 2 * k2 + 2, bass_ts(n, 512)],
                        start=(k2 == 0),
                        stop=(k2 == 1),
                        perf_mode=_dr(),
                    )
                nc.vector.tensor_scalar_add(
                    qk_sb[:, t, bass_ts(n, 512)], acc_a, b_qk[:, t : t + 1]
                )
                nc.vector.tensor_scalar_add(
                    qk_sb[:, t2, bass_ts(n, 512)], acc_b, b_qk[:, t2 : t2 + 1]
                )

        qk_proj(0, 1)

        # ---- V projection: v_sb[:, st, f] = 64*(V rows s-tile st) ----
        v_sb = vp.tile([128, ST, F], BF16)
        for stp in range(ST // 2):
            st, st2 = 2 * stp, 2 * stp + 1
            acc_a = pp.tile([128, 512], F32, name="va", tag="pp")
            acc_b = pp.tile([128, 512], F32, name="vb", tag="pp")
            for k2 in range(2):
                nc.tensor.matmul(
                    acc_a,
                    x8[:, 2 * k2 : 2 * k2 + 2, bass_ts(st, 128)],
                    wv8_sb[:, 2 * k2 : 2 * k2 + 2, :],
                    start=(k2 == 0),
                    stop=(k2 == 1),
                    perf_mode=_dr(),
                )
                nc.tensor.matmul(
                    acc_b,
                    x8[:, 2 * k2 : 2 * k2 + 2, bass_ts(st2, 128)],
                    wv8_sb[:, 2 * k2 : 2 * k2 + 2, :],
                    start=(k2 == 0),
                    stop=(k2 == 1),
                    perf_mode=_dr(),
                )
            nc.vector.tensor_add(v_sb[:, st, :], acc_a, b_v2[:, 0:F])
            nc.vector.tensor_add(v_sb[:, st2, :], acc_b, b_v2[:, 0:F])

        # ---- attention per head; next head's QK projection is emitted after
        # each head so its PE work fills the ACT-bound softmax phase ----
        resT_sb = rp.tile([128, NH, S], FP8)  # res^T * 64: (d, head, i)
        for h in range(NH):
            racc = [pr.tile([128, 512], F32, name=f"racc{n}", tag="racc") for n in range(NT)]
            for jtp in range(ST // 2):
                e2 = ep.tile([128, 2, S], FP8, name="e2", tag="e2")
                v_sc2 = vsp.tile([128, 2, DK], FP8, name="vsc", tag="vsc")
                for half in range(2):
                    jt = 2 * jtp + half
                    ssum = smallp.tile([128, 2], F32, name="ssum", tag="ssum")
                    # scores S^T[j, i] for one j-tile: [128, 1024] PSUM
                    # (2 banks); one exp pass over both halves, softmax
                    # denominator via accum_out. psum holds 256*s.
                    sacc = ps.tile([128, S], F32, name="sacc", tag="sacc")
                    for n in range(NT):
                        nc.tensor.matmul(
                            sacc[:, bass_ts(n, 512)],
                            qk_sb[:, 2 * h + 1, bass_ts(jt, 128)],
                            qk_sb[:, 2 * h, bass_ts(n, 512)],
                            start=True,
                            stop=True,
                        )
                    nc.scalar.activation(
                        out=e2[:, half, :],
                        in_=sacc,
                        func=EXP,
                        scale=SCALE / (WQK_SCALE * WQK_SCALE),
                        bias=b_esh,
                        accum_out=ssum[:, 0:1],
                    )
                    nc.vector.reciprocal(ssum[:, 1:2], ssum[:, 0:1])
                    nc.vector.tensor_scalar_mul(
                        v_sc2[:, half, :],
                        v_sb[:, jt, bass_ts(h, DK)],
                        ssum[:, 1:2],
                    )
                for n in range(NT):
                    nc.tensor.matmul(
                        racc[n],
                        v_sc2,
                        e2[:, :, bass_ts(n, 512)],
                        start=(jtp == 0),
                        stop=(jtp == ST // 2 - 1),
                        perf_mode=_dr(),
                    )
            for n in range(NT):
                nc.vector.tensor_copy(
                    resT_sb[:, h, bass_ts(n, 512)], racc[n]
                )
            if h + 1 < NH:
                qk_proj(2 * h + 2, 2 * h + 3)

        # ---- output projection + bias + residual ----
        for ct in range(KT):
            out_t = op.tile([128, S], F32)
            acc = ps.tile([128, S], F32, name="oacc", tag="sacc")
            for f2 in range(2):
                for n in range(NT):
                    nc.tensor.matmul(
                        acc[:, bass_ts(n, 512)],
                        wo8_sb[:, 2 * f2 : 2 * f2 + 2, bass_ts(ct, 128)],
                        resT_sb[:, 2 * f2 : 2 * f2 + 2, bass_ts(n, 512)],
                        start=(f2 == 0),
                        stop=(f2 == 1),
                        perf_mode=_dr(),
                    )
            # per-half epilogue+store so the tail drains at 512 granularity
            # (first half's DMA overlaps second half's ops). One fused op:
            # out = acc/1024 + (x + b_out), off the busy ACT engine.
            # (must be DVE: GPSIMD cannot read PSUM)
            for n in range(NT):
                nc.vector.scalar_tensor_tensor(
                    out_t[:, bass_ts(n, 512)],
                    acc[:, bass_ts(n, 512)],
                    1.0 / (WV_SCALE * WO_SCALE),
                    xb[:, ct, bass_ts(n, 512)],
                    mybir.AluOpType.mult,
                    mybir.AluOpType.add,
                )
                nc.sync.dma_start(
                    out=out_d[b, bass_ts(ct, 128), bass_ts(n, 512)],
                    in_=out_t[:, bass_ts(n, 512)],
                )


def _dr():
    from concourse import mybir

    return mybir.MatmulPerfMode.DoubleRow


def bass_ts(i, size):
    import concourse.bass as bass

    return bass.ts(i, size)


def _prep_inputs(x, w_proj, b_proj, w_out, b_out):
    """Host-side reshaping into the layouts the kernel expects."""
    x_f = np.ascontiguousarray(x.reshape(B, C, S), dtype=np.float32)
    wT = np.asarray(w_proj, dtype=np.float32).T  # [C, 3*F], f = h*384 + j
    w_qkT = WQK_SCALE * np.concatenate(
        [wT[:, h * 384 : h * 384 + 256] for h in range(NH)], axis=1
    )  # [C, 2F]; col tile t=2h -> q_h, t=2h+1 -> k_h
    w_vT = WV_SCALE * np.concatenate(
        [wT[:, h * 384 + 256 : h * 384 + 384] for h in range(NH)], axis=1
    )  # [C, F]
    w_outT = WO_SCALE * np.ascontiguousarray(np.asarray(w_out, dtype=np.float32).T)
    b_proj = np.asarray(b_proj, dtype=np.float32)
    b_qk = WQK_SCALE * np.stack(
        [
            b_proj[h * 384 + half * 128 : h * 384 + half * 128 + 128]
            for h in range(NH)
            for half in range(2)
        ],
        axis=1,
    )  # [128, 2*NH], col t matches qk tile order
    b_v = WV_SCALE * np.concatenate(
        [b_proj[h * 384 + 256 : h * 384 + 384] for h in range(NH)]
    )  # [F]
    b_v_bcast = np.broadcast_to(np.concatenate([b_v, b_v]), (128, 2 * F))
    b_out_t = np.asarray(b_out, dtype=np.float32).reshape(KT, 128).T  # [128, KT]
    esh = np.full((128, 1), ESHIFT, dtype=np.float32)
    bias = np.ascontiguousarray(
        np.concatenate([b_qk, b_v_bcast, b_out_t, esh], axis=1), dtype=np.float32
    )  # [128, 2*NH + 2*F + KT + 1]
    return x_f, np.ascontiguousarray(w_qkT), np.ascontiguousarray(w_vT), w_outT, bias


def kernel(x, w_proj, b_proj, w_out, b_out, n_heads):
    from concourse.bass_utils import run_bass_kernel_spmd

    assert int(n_heads) == NH
    x_f, w_qkT, w_vT, w_outT, bias = _prep_inputs(x, w_proj, b_proj, w_out, b_out)

    if "nc" not in _CACHE:
        _CACHE["nc"] = _build()
    nc = _CACHE["nc"]

    in_maps = [
        {
            "x": np.ascontiguousarray(x_f[c * BL : (c + 1) * BL]),
            "w_qkT": w_qkT,
            "w_vT": w_vT,
            "w_outT": w_outT,
            "bias": bias,
        }
        for c in range(NCORES)
    ]
    res = run_bass_kernel_spmd(nc, in_maps, list(range(NCORES)))
    out = np.concatenate([res.results[c]["out"] for c in range(NCORES)], axis=0)
    return out.reshape(B, C, 32, 32)


# revision 13
# speedup vs baseline: 1.4147x; 1.1898x over previous
"""AttentionBlock Trainium2 kernel (fp8-DoubleRow edition).

Reference computation (B=16, C=512, H=W=32, n_heads=4, d_k=128):
    xs   = x.reshape(B,C,S).T            # [B, S, C],  S = 1024
    qkv  = xs @ w_proj.T + b_proj        # [B, S, 1536]
    S_   = einsum('bihd,bjhd->bijh', q, k) * d_k**-0.5
    attn = softmax(S_, axis=1)           # over the QUERY axis i (source quirk)
    res  = einsum('bijh,bjhd->bihd', attn, v)
    out  = res @ w_out.T + b_out + xs    # residual
    return out.T.reshape(B, C, H, W)

Strategy: data-parallel over batch, 2 batches per core on 8 cores. Transposed
layouts avoid on-device transposes.

Precision: the output is dominated by the fp32 residual xs (+b_out); the
attention path contributes only ~8% of the output magnitude, so it tolerates
fp8. All projections and the AV contraction run as fp8e4 DoubleRow matmuls
(K=256 per pass, ~2 output cols/cycle: measured ~207-244ns vs 270ns for a
bf16 K=128 matmul). Scores stay bf16 (contraction is only d_k=128, DoubleRow
can't help, and it keeps exp() inputs accurate). fp8 range handling:
  wqk *= 16   -> qk_sb holds 16*(q|k) in bf16; exp scale absorbs the 256
  wv  *= 64   -> v_sb holds 64*v; v_sc = 64*v/Z stays in fp8-normal range
  wo  *= 16   -> out-proj psum = 1024*(res@wo.T); epilogue applies 1/1024
  exp bias=-2 (softmax shift-invariant) keeps e^s in [~0.007, ~40] for fp8
The residual + b_out add is one fp32 scalar_tensor_tensor on DVE against a
precomputed xb = x + b_out.

Scheduling: the softmax (ACT) is the second-busiest engine after PE, so all
PE-only projection work is interleaved into the ACT-bound attention phases:
next-head QK projections after each head, the NEXT batch's QK01+V projection
during the last head, and batch 0's output projection inside batch 1's head
phases. PSUM accumulation is emitted bank-interleaved (A,B,A,B) because
back-to-back same-bank accumulating matmuls measure ~+60ns each.
"""
import sys

for _p in (
    "/opt/trn_rl_repo",
    "/root/.axon_site",
    "/root/.axon_site/_ro/trn_rl_repo",
    "/root/.axon_site/_ro/pypackages",
):
    if _p not in sys.path:
        sys.path.append(_p)

import numpy as np

B = 16
C = 512
S = 1024  # H*W
NH = 4
DK = 128
F = NH * DK  # 512
NCORES = 8
BL = B // NCORES  # batches per core
KT = C // 128  # 4  contraction tiles over channels
ST = S // 128  # 8  seq tiles
NT = S // 512  # 2  free-dim chunks of 512
SCALE = float(DK) ** -0.5
WQK_SCALE = 16.0  # host pre-scale on w_qkT/b_qk -> scores psum = 256*s
WV_SCALE = 64.0  # host pre-scale on w_vT/b_v -> racc = 64*res
WO_SCALE = 16.0  # host pre-scale on w_outT -> out psum = 1024*out_attn
ESHIFT = -2.0  # exp(s*scale + ESHIFT): softmax-invariant fp8 range shift

_CACHE: dict = {}


def _build(repeat=1, unroll=1):
    """Build the kernel. repeat>1 wraps the per-call workload in an on-device
    For_i loop — used only for timing (amortizes the ~10ms axon dispatch).
    unroll>1 emits the workload N times inline (cost-model analysis only)."""
    import contextlib

    import concourse.tile as tile
    from concourse import bacc, mybir

    F32 = mybir.dt.float32
    BF16 = mybir.dt.bfloat16
    FP8 = mybir.dt.float8e4
    EXP = mybir.ActivationFunctionType.Exp

    nc = bacc.Bacc("TRN2", debug=False)
    x_d = nc.dram_tensor("x", [BL, C, S], F32, kind="ExternalInput").ap()
    wqk_d = nc.dram_tensor("w_qkT", [C, 2 * F], F32, kind="ExternalInput").ap()
    wv_d = nc.dram_tensor("w_vT", [C, F], F32, kind="ExternalInput").ap()
    wo_d = nc.dram_tensor("w_outT", [F, C], F32, kind="ExternalInput").ap()
    bias_d = nc.dram_tensor("bias", [128, 2 * NH + 2 * F + KT + 1], F32, kind="ExternalInput").ap()
    out_d = nc.dram_tensor("out", [BL, C, S], F32, kind="ExternalOutput").ap()

    wqk_r = wqk_d.rearrange("(k p) m -> p k m", p=128)
    wv_r = wv_d.rearrange("(k p) m -> p k m", p=128)
    wo_r = wo_d.rearrange("(k p) m -> p k m", p=128)

    with tile.TileContext(nc) as tc:
        with (
            tc.tile_pool(name="const", bufs=1) as constp,
            tc.tile_pool(name="stage", bufs=1) as stagep,
            tc.tile_pool(name="xp", bufs=2) as xp,
            tc.tile_pool(name="qkp", bufs=2) as qkp,
            tc.tile_pool(name="vp", bufs=2) as vp,
            tc.tile_pool(name="ep", bufs=3) as ep,
            tc.tile_pool(name="rp", bufs=2) as rp,
            tc.tile_pool(name="op", bufs=2) as op,
            tc.tile_pool(name="small", bufs=16) as smallp,
            tc.tile_pool(name="vs", bufs=8) as vsp,
            # psum: pp = [128,512]x2 shared by qk/v/out projections;
            # ps = [128,1024]x2 for score tiles; pr = [128,512]x2 for the
            # per-head AV accumulators. 2+4+2 = 8 banks.
            tc.tile_pool(name="pp", bufs=2, space="PSUM") as pp,
            tc.tile_pool(name="ps", bufs=2, space="PSUM") as ps,
            tc.tile_pool(name="pr", bufs=2, space="PSUM") as pr,
        ):
            # ---- constants: load fp32, convert once to fp8 ----
            wqk8_sb = constp.tile([128, KT, 2 * F], FP8)
            wv8_sb = constp.tile([128, KT, F], FP8)
            wo8_sb = constp.tile([128, KT, C], FP8)
            bias_sb = constp.tile([128, 2 * NH + 2 * F + KT + 1], F32)
            x_sbs = [xp.tile([128, KT, S], F32, name=f"x{b}", tag="x") for b in range(BL)]
            x8_sbs = [xp.tile([128, KT, S], FP8, name=f"x8{b}", tag="x8") for b in range(BL)]
            # xb = x + b_out (residual + out bias pre-added once; the out-proj
            # epilogue is then a single scalar_tensor_tensor per half)
            xb_sbs = [xp.tile([128, KT, S], F32, name=f"xb{b}", tag="xb") for b in range(BL)]

            wqk_st = stagep.tile([128, KT, 2 * F], F32, name="wst", tag="wst")
            for k in range(KT):
                nc.sync.dma_start(out=x_sbs[0][:, k, :], in_=x_d[0, bass_ts(k, 128), :])
                nc.sync.dma_start(out=wqk_st[:, k, :], in_=wqk_r[:, k, :])
            nc.vector.tensor_copy(x8_sbs[0], x_sbs[0])
            nc.gpsimd.tensor_copy(wqk8_sb, wqk_st)
            wv_st = stagep.tile([128, KT, F], F32, name="wst", tag="wst")
            nc.sync.dma_start(out=wv_st, in_=wv_r)
            nc.gpsimd.tensor_copy(wv8_sb, wv_st)
            nc.sync.dma_start(out=bias_sb, in_=bias_d)
            for b in range(1, BL):
                for k in range(KT):
                    nc.sync.dma_start(out=x_sbs[b][:, k, :], in_=x_d[b, bass_ts(k, 128), :])
                nc.vector.tensor_copy(x8_sbs[b], x_sbs[b])
            wo_st = stagep.tile([128, KT, C], F32, name="wst", tag="wst")
            nc.sync.dma_start(out=wo_st, in_=wo_r)
            nc.gpsimd.tensor_copy(wo8_sb, wo_st)
            for b in range(BL):
                for k in range(KT):
                    nc.gpsimd.tensor_scalar_add(
                        xb_sbs[b][:, k, :],
                        x_sbs[b][:, k, :],
                        bias_sb[:, 2 * NH + 2 * F + k : 2 * NH + 2 * F + k + 1],
                    )
            b_qk = bias_sb[:, 0 : 2 * NH]  # per-partition bias per qk f-tile (x16)
            b_v2 = bias_sb[:, 2 * NH : 2 * NH + 2 * F]  # v bias doubled (x64)
            b_esh = bias_sb[:, 2 * NH + 2 * F + KT :]  # ESHIFT constant column

            env = dict(
                nc=nc, qkp=qkp, vp=vp, ep=ep, rp=rp, op=op, smallp=smallp,
                vsp=vsp, pp=pp, ps=ps, pr=pr, wqk8_sb=wqk8_sb, wv8_sb=wv8_sb,
                wo8_sb=wo8_sb, b_qk=b_qk, b_v2=b_v2, b_esh=b_esh, out_d=out_d,
                x8_sbs=x8_sbs, xb_sbs=xb_sbs, F32=F32, BF16=BF16, FP8=FP8,
                EXP=EXP, mybir=mybir,
            )
            # software-pipeline prologue: batch 0's QK01 + V projections
            pend = {0: _proj_phase(env, 0)}

            rep_ctx = (
                tc.For_i(0, repeat, 1) if repeat > 1 else contextlib.nullcontext()
            )
            with rep_ctx:
                for _u in range(unroll):
                    _batches(env, pend)

    nc.compile()
    return nc


def _proj_phase(env, b):
    """QK f-tiles 0,1 + full V projection for batch b. Returns (qk_sb, v_sb)."""
    nc = env["nc"]
    F32, BF16 = env["F32"], env["BF16"]
    x8 = env["x8_sbs"][b]
    pp, b_qk, b_v2 = env["pp"], env["b_qk"], env["b_v2"]
    wv8_sb = env["wv8_sb"]

    qk_sb = env["qkp"].tile([128, 2 * NH, S], BF16, name=f"qk{b}", tag="qk")
    _qk_proj(env, x8, qk_sb, 0, 1)
    v_sb = env["vp"].tile([128, ST, F], BF16, name=f"v{b}", tag="v")
    for stp in range(ST // 2):
        st, st2 = 2 * stp, 2 * stp + 1
        acc_a = pp.tile([128, 512], F32, name="va", tag="pp")
        acc_b = pp.tile([128, 512], F32, name="vb", tag="pp")
        for k2 in range(2):
            nc.tensor.matmul(
                acc_a,
                x8[:, 2 * k2 : 2 * k2 + 2, bass_ts(st, 128)],
                wv8_sb[:, 2 * k2 : 2 * k2 + 2, :],
                start=(k2 == 0),
                stop=(k2 == 1),
                perf_mode=_dr(),
            )
            nc.tensor.matmul(
                acc_b,
                x8[:, 2 * k2 : 2 * k2 + 2, bass_ts(st2, 128)],
                wv8_sb[:, 2 * k2 : 2 * k2 + 2, :],
                start=(k2 == 0),
                stop=(k2 == 1),
                perf_mode=_dr(),
            )
        nc.vector.tensor_add(v_sb[:, st, :], acc_a, b_v2[:, 0:F])
        nc.vector.tensor_add(v_sb[:, st2, :], acc_b, b_v2[:, 0:F])
    return qk_sb, v_sb


def _qk_proj(env, x8, qk_sb, t, t2):
    # Q^T/K^T f-tiles t,t2: qk_sb[:, t, s] = 16*w_qkT[:, t].T @ x
    # Two DoubleRow k-blocks (channels 0:256, 256:512), A/B bank
    # interleaved so same-bank accumulating MMs are 2 apart.
    nc = env["nc"]
    F32 = env["F32"]
    pp, wqk8_sb, b_qk = env["pp"], env["wqk8_sb"], env["b_qk"]
    for n in range(NT):
        acc_a = pp.tile([128, 512], F32, name="qka", tag="pp")
        acc_b = pp.tile([128, 512], F32, name="qkb", tag="pp")
        for k2 in range(2):
            nc.tensor.matmul(
                acc_a,
                wqk8_sb[:, 2 * k2 : 2 * k2 + 2, bass_ts(t, 128)],
                x8[:, 2 * k2 : 2 * k2 + 2, bass_ts(n, 512)],
                start=(k2 == 0),
                stop=(k2 == 1),
                perf_mode=_dr(),
            )
            nc.tensor.matmul(
                acc_b,
                wqk8_sb[:, 2 * k2 : 2 * k2 + 2, bass_ts(t2, 128)],
                x8[:, 2 * k2 : 2 * k2 + 2, bass_ts(n, 512)],
                start=(k2 == 0),
                stop=(k2 == 1),
                perf_mode=_dr(),
            )
        nc.vector.tensor_scalar_add(
            qk_sb[:, t, bass_ts(n, 512)], acc_a, b_qk[:, t : t + 1]
        )
        nc.vector.tensor_scalar_add(
            qk_sb[:, t2, bass_ts(n, 512)], acc_b, b_qk[:, t2 : t2 + 1]
        )


def _out_proj_chunk(env, b, ct, resT_sb):
    """Output projection c-tile ct for batch b + fused epilogue + store."""
    nc = env["nc"]
    F32, mybir = env["F32"], env["mybir"]
    pp, wo8_sb, out_d = env["pp"], env["wo8_sb"], env["out_d"]
    xb = env["xb_sbs"][b]

    out_t = env["op"].tile([128, S], F32, name="ot", tag="ot")
    accs = [pp.tile([128, 512], F32, name=f"oa{n}", tag="pp") for n in range(NT)]
    for f2 in range(2):
        for n in range(NT):
            nc.tensor.matmul(
                accs[n],
                wo8_sb[:, 2 * f2 : 2 * f2 + 2, bass_ts(ct, 128)],
                resT_sb[:, 2 * f2 : 2 * f2 + 2, bass_ts(n, 512)],
                start=(f2 == 0),
                stop=(f2 == 1),
                perf_mode=_dr(),
            )
    # fused epilogue: out = acc/1024 + (x + b_out), then store per half.
    # (must be DVE: GPSIMD cannot read PSUM)
    for n in range(NT):
        nc.vector.scalar_tensor_tensor(
            out_t[:, bass_ts(n, 512)],
            accs[n],
            1.0 / (WV_SCALE * WO_SCALE),
            xb[:, ct, bass_ts(n, 512)],
            mybir.AluOpType.mult,
            mybir.AluOpType.add,
        )
        nc.sync.dma_start(
            out=out_d[b, bass_ts(ct, 128), bass_ts(n, 512)],
            in_=out_t[:, bass_ts(n, 512)],
        )


def _batches(env, pend):
    """One full workload pass (both batches), software-pipelined.

    Batch b's attention phases interleave: next-head QK tiles (h0-h2), the
    NEXT batch's QK01+V projection (h3), and for b=1 the PREVIOUS batch's
    output projection (one c-tile per head phase). Batch 1's own output
    projection runs at the end (the only PE-only stretch left).
    """
    nc = env["nc"]
    F32, FP8, EXP = env["F32"], env["FP8"], env["EXP"]
    ep, rp, smallp, vsp = env["ep"], env["rp"], env["smallp"], env["vsp"]
    ps, pr = env["ps"], env["pr"]
    b_esh = env["b_esh"]

    resT = {}
    for b in range(BL):
        qk_sb, v_sb = pend.pop(b)
        x8_next = env["x8_sbs"][(b + 1) % BL]

        resT_sb = rp.tile([128, NH, S], FP8, name=f"resT{b}", tag="resT")
        resT[b] = resT_sb
        for h in range(NH):
            racc = [pr.tile([128, 512], F32, name=f"racc{n}", tag="racc") for n in range(NT)]
            for jtp in range(ST // 2):
                e2 = ep.tile([128, 2, S], FP8, name="e2", tag="e2")
                v_sc2 = vsp.tile([128, 2, DK], FP8, name="vsc", tag="vsc")
                for half in range(2):
                    jt = 2 * jtp + half
                    ssum = smallp.tile([128, 2], F32, name="ssum", tag="ssum")
                    # scores S^T[j, i] for one j-tile: [128, 1024] PSUM
                    # (2 banks); one exp pass over both halves, softmax
                    # denominator via accum_out. psum holds 256*s.
                    sacc = ps.tile([128, S], F32, name="sacc", tag="sacc")
                    for n in range(NT):
                        nc.tensor.matmul(
                            sacc[:, bass_ts(n, 512)],
                            qk_sb[:, 2 * h + 1, bass_ts(jt, 128)],
                            qk_sb[:, 2 * h, bass_ts(n, 512)],
                            start=True,
                            stop=True,
                        )
                    nc.scalar.activation(
                        out=e2[:, half, :],
                        in_=sacc,
                        func=EXP,
                        scale=SCALE / (WQK_SCALE * WQK_SCALE),
                        bias=b_esh,
                        accum_out=ssum[:, 0:1],
                    )
                    nc.vector.reciprocal(ssum[:, 1:2], ssum[:, 0:1])
                    nc.vector.tensor_scalar_mul(
                        v_sc2[:, half, :],
                        v_sb[:, jt, bass_ts(h, DK)],
                        ssum[:, 1:2],
                    )
                for n in range(NT):
                    nc.tensor.matmul(
                        racc[n],
                        v_sc2,
                        e2[:, :, bass_ts(n, 512)],
                        start=(jtp == 0),
                        stop=(jtp == ST // 2 - 1),
                        perf_mode=_dr(),
                    )
            for n in range(NT):
                nc.vector.tensor_copy(
                    resT_sb[:, h, bass_ts(n, 512)], racc[n]
                )
            # PE-only filler for this ACT-bound phase:
            if h + 1 < NH:
                _qk_proj(env, env["x8_sbs"][b], qk_sb, 2 * h + 2, 2 * h + 3)
            else:
                # last head: project the next batch (next iteration's b=0
                # when b is the last batch — recomputed there, harmless)
                pend[(b + 1) % BL] = _proj_phase(env, (b + 1) % BL)
            if b == BL - 1:
                # batch 0's output projection, one c-tile per head phase
                _out_proj_chunk(env, 0, h, resT[0])

    # ---- batch 1's output projection (tail) ----
    for ct in range(KT):
        _out_proj_chunk(env, BL - 1, ct, resT[BL - 1])


def _dr():
    from concourse import mybir

    return mybir.MatmulPerfMode.DoubleRow


def bass_ts(i, size):
    import concourse.bass as bass

    return bass.ts(i, size)


def _prep_inputs(x, w_proj, b_proj, w_out, b_out):
    """Host-side reshaping into the layouts the kernel expects."""
    x_f = np.ascontiguousarray(x.reshape(B, C, S), dtype=np.float32)
    wT = np.asarray(w_proj, dtype=np.float32).T  # [C, 3*F], f = h*384 + j
    w_qkT = WQK_SCALE * np.concatenate(
        [wT[:, h * 384 : h * 384 + 256] for h in range(NH)], axis=1
    )  # [C, 2F]; col tile t=2h -> q_h, t=2h+1 -> k_h
    w_vT = WV_SCALE * np.concatenate(
        [wT[:, h * 384 + 256 : h * 384 + 384] for h in range(NH)], axis=1
    )  # [C, F]
    w_outT = WO_SCALE * np.ascontiguousarray(np.asarray(w_out, dtype=np.float32).T)
    b_proj = np.asarray(b_proj, dtype=np.float32)
    b_qk = WQK_SCALE * np.stack(
        [
            b_proj[h * 384 + half * 128 : h * 384 + half * 128 + 128]
            for h in range(NH)
            for half in range(2)
        ],
        axis=1,
    )  # [128, 2*NH], col t matches qk tile order
    b_v = WV_SCALE * np.concatenate(
        [b_proj[h * 384 + 256 : h * 384 + 384] for h in range(NH)]
    )  # [F]
    b_v_bcast = np.broadcast_to(np.concatenate([b_v, b_v]), (128, 2 * F))
    b_out_t = np.asarray(b_out, dtype=np.float32).reshape(KT, 128).T  # [128, KT]
    esh = np.full((128, 1), ESHIFT, dtype=np.float32)
    bias = np.ascontiguousarray(
        np.concatenate([b_qk, b_v_bcast, b_out_t, esh], axis=1), dtype=np.float32
    )  # [128, 2*NH + 2*F + KT + 1]
    return x_f, np.ascontiguousarray(w_qkT), np.ascontiguousarray(w_vT), w_outT, bias


def kernel(x, w_proj, b_proj, w_out, b_out, n_heads):
    from concourse.bass_utils import run_bass_kernel_spmd

    assert int(n_heads) == NH
    x_f, w_qkT, w_vT, w_outT, bias = _prep_inputs(x, w_proj, b_proj, w_out, b_out)

    if "nc" not in _CACHE:
        _CACHE["nc"] = _build()
    nc = _CACHE["nc"]

    in_maps = [
        {
            "x": np.ascontiguousarray(x_f[c * BL : (c + 1) * BL]),
            "w_qkT": w_qkT,
            "w_vT": w_vT,
            "w_outT": w_outT,
            "bias": bias,
        }
        for c in range(NCORES)
    ]
    res = run_bass_kernel_spmd(nc, in_maps, list(range(NCORES)))
    out = np.concatenate([res.results[c]["out"] for c in range(NCORES)], axis=0)
    return out.reshape(B, C, 32, 32)
